# revision 19
# baseline (speedup 1.0000x reference)
"""Trainium2 Bass kernel for the CLC block (grouped 3x3 conv -> BN+ReLU ->
grouped 1x1 conv -> BN+ReLU, twice).

Sharding: pure data parallel, batch 32 -> 4 samples per core on 8 cores.

Per-core design (f16 storage/matmul, f32 PSUM accumulate):
  - HBM I/O is f16: the host pre-converts x (the kernel quantizes to f16
    on-chip anyway) and post-converts y back to f32 (~5e-4 rounding,
    far below the 2e-2 gate).  Halves DMA traffic both directions.
  - Channel-major layout: [128 channel partitions, pixels] per
    128-channel half; 7 pixel regions of 8 rows (448 px = one PSUM bank).
  - gconv3x3 is block-diagonal at 64-channel granularity in the g-major
    output layout (out pos m = 4*g_loc + i).  Each tap is computed with the
    two 64x64 diagonal blocks loaded into all four PE-array quadrants:
      T0 (rows 0:64,  psum 0:64)   D0 -> even regions outputs 0:64
      T10(rows 64:128,psum 64:128) D1 -> even regions outputs 64:128
      T2 (rows 0:64,  psum 64:128) D0 -> odd regions outputs 0:64
      T8 (rows 64:128,psum 0:64)   D1 -> odd regions outputs 64:128
    Odd ("rotated") regions' PSUM banks hold the two 64-channel halves
    swapped -- downstream pw weights compensate, no swap copies needed.
    The four quadrant tiles stream concurrently (~4 distinct rhs streams
    sustained, which saturates the PE array's MAC capacity); emission
    alternates tiles because MATMUL starts are pc-monotone.
  - Tap-outer ordering over sets of regions (PSUM banks) makes
    consecutive matmuls share lhsT; a post-schedule pass prunes the
    redundant LDWEIGHTS the legalizer inserts.
  - pw 1x1 conv: per region, 2 accumulating K=128 full-array matmuls
    (one per g-major input half), lhsT variant chosen by region rotation.
  - BN + conv-bias fold into matmul weights host-side; each stage needs a
    single bias+ReLU pass evacuating PSUM->SBUF (ACT/DVE alternating).
  - Input pad-copies are emitted AFTER the first gconv stage so the
    ACT/DVE queues don't stall on the input DMA with relu work behind
    them; gconv emits halves interleaved (h0s0, h1s0, h0s1, h1s1) so pw
    can start as soon as the PE drains.

Measured on HW (hwloop-contrast wall timing, best of trials): ~166 us
per batch vs 199 us for the session-start baseline.  Ablations: with
relu+DMA removed the floor is ~164 us, i.e. the kernel is PE-matmul
bound; the gconv streams 9 taps x 2 bands over 4 concurrent 64-row
streams and cannot go faster without reducing streamed columns.

Dead ends measured this session (do not retry naively):
  - 32x32 tiling (_G32): 16 tiles do NOT stream 16 distinct rhs
    concurrently; ~one stream per 32-col group at best.  576 us.
  - Widening gconv LDWEIGHTS to 128 cols (FWL): a full-width load
    conflicts with every in-flight quadrant stream -> serializes the
    array.  396 us.  Narrow per-quadrant LDWEIGHTS pipeline fine.
  - _TAP8 (all regions in one tap-outer loop): 182 us -- longer relu
    drain tail beats the LDWEIGHTS savings.
  - pw in fp8 DoubleRow (_PW8): ~4e-2 rel err, over the 2e-2 gate
    (prior session).
  - Region-pair matmuls: tried BOTH a gapped 4D ifmap (regions r, r+2
    via AP surgery, _PAIR flag) and a contiguous 16-row window (adjacent
    regions sharing rho) with a 3D strided 2-bank PSUM out.  Walrus
    rejects both ("generates invalid ISA instruction"): the matmul PSUM
    out AP must be contiguous within ONE bank, so N > 512 f32 per MM is
    architecturally impossible.  MM count cannot be reduced below one
    per (region, band).
  - PSUM tile-pool tags allocate STATICALLY (no liveness reuse): all
    stages must share the four 2-bank "pp" tags (16KB = all 8 banks);
    gconv2/pw address regions as 512-f32 offset halves of those tiles.
"""

import numpy as np

B, C, H, W = 32, 256, 56, 56
EPS = 1e-5
N_CORES = 8
BPC = B // N_CORES  # samples per core
HP, WP = H + 2, W + 2  # padded spatial
NPIX = H * W
NPAD = HP * WP
ROWS_PER_TILE = 8
NREG = H // ROWS_PER_TILE  # 7 pixel regions
TILE_PX = ROWS_PER_TILE * W  # 448 (fits a 512-f32 PSUM bank)


# ---------------------------------------------------------------------------
# Host-side weight preparation
# ---------------------------------------------------------------------------

def _bn_fold(bg, bb, bm, bv):
    inv = bg / np.sqrt(bv + EPS)
    return inv, bb - bm * inv


def prepare_weights(inp):
    """Returns (wg, wp [128, 2*2*2*4*128], bias [128, 24]), all f16/f32.

    Rotation rho (0..3): a PSUM bank in rotation rho holds natural output
    position p' at partition p = (p' + 32*rho) % 128 -- i.e.
    psum = roll(nat, 32*rho).  The 64x64 scheme uses rho in {0, 2}; the
    32x32 scheme uses rho = region % 4.

    wg (G32 off): [128, ((l*2+h)*9 + t)*64 + m64] diag-block lhsT; rows
      0:64 = D0 (outputs 0:64), rows 64:128 = D1.
    wg (G32 on):  [128, ((l*2+h)*9 + t)*32 + m32]; rows 32q:32q+32 = Dq
      (outputs 32q:32q+32 of the half).
    wp[k, ((((l*2+Hh)*2 + A)*4 + rho)*128 + m]: lhsT for pw layer l, output
      half Hh (natural), g-major input half A, input rotation rho.
    bias cols: gconv = rho*4 + 2*l + h (0..15), pw = 16 + 2*l + Hh.
    """
    f32 = np.float32
    wg_full = np.zeros((128, 2, 2, 9, 128), f32)
    wp_nat = np.zeros((128, 2, 2, 2, 128), f32)
    bias = np.zeros((128, 24), f32)

    gconv_params = [
        (inp["w1"], inp["b1"], inp["bn1a_g"], inp["bn1a_b"], inp["bn1a_m"], inp["bn1a_v"]),
        (inp["w2"], inp["b2"], inp["bn2a_g"], inp["bn2a_b"], inp["bn2a_m"], inp["bn2a_v"]),
    ]
    pw_params = [
        (inp["pw1"], inp["pb1"], inp["bn1b_g"], inp["bn1b_b"], inp["bn1b_m"], inp["bn1b_v"]),
        (inp["pw2"], inp["pb2"], inp["bn2b_g"], inp["bn2b_b"], inp["bn2b_m"], inp["bn2b_v"]),
    ]

    for l, (w, bcv, bg, bb, bm, bv) in enumerate(gconv_params):
        w = np.asarray(w, f32)
        inv, shift = _bn_fold(np.asarray(bg, f32), np.asarray(bb, f32),
                              np.asarray(bm, f32), np.asarray(bv, f32))
        bconv = np.asarray(bcv, f32).reshape(256)  # index i*64+g
        beff = bconv * inv + shift  # natural order o
        for h in range(2):
            bnat = np.zeros(128, f32)
            for m in range(128):
                g = 32 * h + m // 4
                i = m % 4
                o = i * 64 + g
                for kk in range(4):
                    k = 4 * (g - 32 * h) + kk
                    for t in range(9):
                        wg_full[k, l, h, t, m] = w[i, g, kk, t // 3, t % 3] * inv[o]
                bnat[m] = beff[o]
            for rho in range(4):
                bias[:, rho * 4 + 2 * l + h] = np.roll(bnat, 32 * rho)

    # extract diagonal blocks: rows of band b -> output columns of band b
    if _G32[0]:
        # wide layout: per (l, h, tap) a 128-col block holding the band's
        # 32x32 diag block tiled 4x horizontally, so one 128-col LDWEIGHTS
        # (FWL-eligible) loads all four col-rects of the row band
        wg = np.zeros((128, 2, 2, 9, 128), f32)
        for b in range(4):
            Db = wg_full[32 * b:32 * b + 32, :, :, :, 32 * b:32 * b + 32]
            for c in range(4):
                wg[32 * b:32 * b + 32, :, :, :, 32 * c:32 * c + 32] = Db
    else:
        # wide layout: per (l, h, tap) a 128-col block; rows 0:64 hold
        # [D0|D0] (tiles T0 and T2 both want D0), rows 64:128 hold
        # [D1|D1] (T8 and T10), so one 128-col FWL LDWEIGHTS per row
        # half loads both col-rects
        wg = np.zeros((128, 2, 2, 9, 128), f32)
        for b in range(2):
            Db = wg_full[64 * b:64 * b + 64, :, :, :, 64 * b:64 * b + 64]
            for c in range(2):
                wg[64 * b:64 * b + 64, :, :, :, 64 * c:64 * c + 64] = Db

    for l, (w, pb, bg, bb, bm, bv) in enumerate(pw_params):
        w = np.asarray(w, f32).reshape(256, 64)
        inv, shift = _bn_fold(np.asarray(bg, f32), np.asarray(bb, f32),
                              np.asarray(bm, f32), np.asarray(bv, f32))
        beff = np.asarray(pb, f32) * inv + shift
        for Hh in range(2):
            for m in range(128):
                c = 128 * Hh + m
                i = c // 64
                for g in range(64):
                    p = 4 * g + i  # global g-major position of input (i, g)
                    A, k = divmod(p, 128)
                    wp_nat[k, l, Hh, A, m] = w[c, g] * inv[c]
                bias[m, 16 + 2 * l + Hh] = beff[c]

    # input-rotation variants: lhsT_rho[p] = lhsT_nat[(p - 32*rho) % 128].
    # Only the variants a scheme uses are materialized (64x64: rho 0 and 2).
    rhos = _RHOS()
    wp = np.stack([np.roll(wp_nat, 32 * rho, axis=0) for rho in rhos],
                  axis=4)  # [128, l, Hh, A, rho-slot, 128]
    return (wg.reshape(128, -1).astype(np.float16),
            wp.reshape(128, 2 * 2 * 2 * len(rhos) * 128).astype(np.float16),
            bias)


def _RHOS():
    return (0, 1, 2, 3) if _G32[0] else (0, 2)


def _rho(r):
    return (r % 4) if _G32[0] else 2 * (r % 2)


def _bias_col(stage, h, rho):
    # stage 0..3 = gconv1, pw1, gconv2, pw2; l = stage // 2
    if stage % 2 == 0:  # gconv
        return rho * 4 + 2 * (stage // 2) + h
    return 16 + 2 * (stage // 2) + h


# ---------------------------------------------------------------------------
# Numpy emulation of the exact kernel dataflow (for validation)
# ---------------------------------------------------------------------------

def emulate(inp):
    wg, wp, bias = prepare_weights(inp)
    bw = 32 if _G32[0] else 64
    wg = wg.astype(np.float32).reshape(128, 2, 2, 9, -1)
    # wide tiled layouts: every bw-col copy is identical; the diag block
    # for band b is any copy of rows b*bw:(b+1)*bw
    wg = wg[:, :, :, :, 0:bw]
    wp = wp.astype(np.float32).reshape(128, 2, 2, 2, len(_RHOS()), 128)
    x = np.asarray(inp["x"], np.float32)
    out = np.zeros_like(x)

    for n in range(B):
        xpad = np.zeros((2, 128, HP, WP), np.float32)
        for h in range(2):
            xpad[h, :, 1:57, 1:57] = x[n, 128 * h:128 * (h + 1)].astype(np.float16)

        def gconv(src_pad, l):
            td = [np.zeros((128, H, W), np.float32) for _ in range(2)]
            for h in range(2):
                for r in range(NREG):
                    rho = _rho(r)
                    r0 = r * ROWS_PER_TILE
                    acc = np.zeros((128, ROWS_PER_TILE, W), np.float32)
                    for tap in range(9):
                        dh, dw = tap // 3, tap % 3
                        rhs = src_pad[h][:, r0 + dh:r0 + dh + ROWS_PER_TILE,
                                         dw:dw + W].reshape(128, -1)
                        nat = np.concatenate([
                            wg[b * bw:(b + 1) * bw, l, h, tap, :].T
                            @ rhs[b * bw:(b + 1) * bw]
                            for b in range(128 // bw)], 0)
                        acc += np.roll(nat, 32 * rho, axis=0).reshape(
                            128, ROWS_PER_TILE, W)
                    bcol = _bias_col(2 * l, h, rho)
                    res = np.maximum(acc + bias[:, bcol][:, None, None], 0.0)
                    td[h][:, r0:r0 + ROWS_PER_TILE] = res.astype(np.float16)
            return td

        def pw(td, l):
            dst = [None, None]
            for Hh in range(2):
                o = np.zeros((128, H, W), np.float32)
                for r in range(NREG):
                    rho = _rho(r)
                    r0 = r * ROWS_PER_TILE
                    acc = np.zeros((128, ROWS_PER_TILE * W), np.float32)
                    for A in range(2):
                        rhs = td[A][:, r0:r0 + ROWS_PER_TILE].reshape(128, -1)
                        acc += wp[:, l, Hh, A, _RHOS().index(rho), :].T @ rhs
                    bcol = _bias_col(2 * l + 1, Hh, rho)
                    res = np.maximum(acc + bias[:, bcol][:, None], 0.0)
                    o[:, r0:r0 + ROWS_PER_TILE] = res.reshape(128, ROWS_PER_TILE, W)
                dst[Hh] = o
            return dst

        t1 = gconv(xpad, 0)
        t2 = pw(t1, 0)
        t2pad = np.zeros((2, 128, HP, WP), np.float32)
        for h in range(2):
            t2pad[h, :, 1:57, 1:57] = t2[h].astype(np.float16)
        t3 = gconv(t2pad, 1)
        y = pw(t3, 1)
        out[n, 0:128] = y[0]
        out[n, 128:256] = y[1]
    return out


# ---------------------------------------------------------------------------
# Bass program
# ---------------------------------------------------------------------------

_CACHED = {}
_REPEAT = [1]
_HWLOOP = [1]
_PW8 = [False]  # pw in fp8 DoubleRow: fast but ~4e-2 rel err -- too lossy
# 32x32 PE tiling measured 249 us/iter vs 173 us for 64x64 on HW (the
# 16 serialized 32-col LDWEIGHTS per tap dominate) -- keep 64x64.
_G32 = [False]
# UNTESTED candidate (census-driven, see memory): single 8-region tap-outer
# emission so one LDWEIGHTS covers both region-sets' matmuls per quadrant
# (-576 LDW/program ~ -30us weight path, vs ~2us/sample extra PSUM boundary
# stalls from holding all 8 banks).  Flip with set_tap8(True) and verify.
_TAP8 = [False]
_PAIR = [False]  # region-pair MMs via AP surgery: walrus rejects the
# 4D gapped ifmap ("invalid ISA instruction") -- PE ifmap APs are 3D max


_SKIP = {"gconv": False, "pw": False, "dma": False, "relu": False}


def set_skip(which, v=True):
    _SKIP[which] = bool(v)


def set_tap8(v):
    _TAP8[0] = bool(v)


def set_pair(v):
    _PAIR[0] = bool(v)


def set_pw8(v):
    _PW8[0] = bool(v)


def set_g32(v):
    _G32[0] = bool(v)


def set_repeat(r):
    _REPEAT[0] = r


def set_hwloop(r):
    _HWLOOP[0] = r


def _build_body(tc, y_ap, x_ap, wg_ap, wp_ap, bias_ap, zeros_ap, repeat=1):
    import concourse.bass as bass  # noqa: F401
    from concourse import mybir

    nc = tc.nc
    f32 = mybir.dt.float32
    f16 = mybir.dt.float16
    f8 = mybir.dt.float8e4
    ADD = mybir.AluOpType.add
    MAX = mybir.AluOpType.max
    RELU = mybir.ActivationFunctionType.Relu
    DR = mybir.MatmulPerfMode.DoubleRow

    ctx = tc._build_ctx

    const = ctx.enter_context(tc.tile_pool(name="const", bufs=1))
    persist = ctx.enter_context(tc.tile_pool(name="persist", bufs=1))
    pspool = ctx.enter_context(tc.tile_pool(name="ps", bufs=1, space="PSUM"))

    bw = 32 if _G32[0] else 64
    wg_sb = const.tile([128, 2 * 2 * 9 * 128], f16, tag="wg", name="wg_sb")
    nrho = len(_RHOS())
    wp_sb = const.tile([128, 2 * 2 * 2 * nrho * 128], f16, tag="wp", name="wp_sb")
    bias_sb = const.tile([128, 24], f32, tag="bias", name="bias_sb")
    nc.sync.dma_start(wg_sb[:], wg_ap)
    nc.sync.dma_start(wp_sb[:], wp_ap)
    nc.sync.dma_start(bias_sb[:], bias_ap)

    # double-buffered padded input: sample n+1's load+pad-copy runs during
    # sample n's compute so the PE never waits on the ACT/DVE copy
    xpad = [[persist.tile([128, NPAD], f16, tag=f"xpad{b}{h}", name=f"xpad{b}{h}")
             for h in range(2)] for b in range(2)]
    xstage = [persist.tile([128, NPIX], f16, tag=f"xstage{h}", name=f"xstage{h}") for h in range(2)]
    r2pad = [persist.tile([128, NPAD], f16, tag=f"r2pad{h}", name=f"r2pad{h}") for h in range(2)]
    td = [persist.tile([128, NPIX], f16, tag=f"td{h}", name=f"td{h}") for h in range(2)]
    td_dst = lambda h, r: td[h][:, r * TILE_PX:(r + 1) * TILE_PX]
    NB = 2
    ysb = [[persist.tile([128, NPIX], f16, tag=f"ysb{b}{h}", name=f"ysb{b}{h}") for h in range(2)] for b in range(NB)]

    def p3(tile_):
        return tile_[:].rearrange("p (a b) -> p a b", b=WP)

    for t in xpad[0] + xpad[1] + r2pad:
        v = p3(t)
        flat = t[:]
        nc.sync.dma_start(flat[:, 0:WP], zeros_ap[:, 0:WP])
        nc.sync.dma_start(flat[:, (HP - 1) * WP:HP * WP], zeros_ap[:, 0:WP])
        nc.sync.dma_start(v[:, 1:HP - 1, 0:1], zeros_ap[:, 0:HP - 2])
        nc.sync.dma_start(v[:, 1:HP - 1, WP - 1:WP], zeros_ap[:, 0:HP - 2])

    def relu_pass(dst, ps, scol, use_act):
        if _SKIP["relu"]:
            return
        if use_act:
            nc.scalar.activation(dst, ps, RELU, bias=bias_sb[:, scol:scol + 1])
        else:
            nc.vector.tensor_scalar(dst, ps, bias_sb[:, scol:scol + 1], 0.0,
                                    op0=ADD, op1=MAX)

    def gconv_stage_g32(src_pads, l):
        # 16 concurrent 32x32 tiles; tap-outer over all 8 regions (8 banks).
        # lhsT for (q, c) is copy c of the 4x-tiled wide block, so the
        # widen_g32_ldweights pass can fuse each band's four 32-col LDW
        # into one 128-col (FWL) load.
        for h in range(2):
            src = p3(src_pads[h])
            wbase = ((l * 2 + h) * 9) * 128
            ps = [pspool.tile([128, TILE_PX], f32, tag=f"ps{j // 4}{j % 4}",
                              name=f"g{l}{h}{j}") for j in range(8)]
            for tap in range(9):
                dh, dw = tap // 3, tap % 3
                wc = wbase + tap * 128
                st, sp = (tap == 0), (tap == 8)
                for q in range(4):
                    for c in range(4):
                        rho = (c - q) % 4
                        Dqc = wg_sb[32 * q:32 * q + 32,
                                    wc + 32 * c:wc + 32 * c + 32]
                        for P in range(2):
                            r0 = (P * 4 + rho) * ROWS_PER_TILE
                            nc.tensor.matmul(
                                ps[P * 4 + rho][32 * c:32 * c + 32, :],
                                lhsT=Dqc,
                                rhs=src[32 * q:32 * q + 32,
                                        r0 + dh:r0 + dh + ROWS_PER_TILE,
                                        dw:dw + W],
                                start=st, stop=sp,
                                tile_position=(32 * q, 32 * c))
            for r in range(8):
                relu_pass(td_dst(h, r), ps[r][:], _bias_col(2 * l, h, _rho(r)),
                          use_act=(r % 2 == 0))

    def gconv_stage_tap8(src_pads, l):
        # one tap-outer loop over all 8 regions: both sets' same-quadrant
        # matmuls are adjacent, so one LDWEIGHTS serves 4 MMs per tap
        for h in range(2):
            src = p3(src_pads[h])
            wbase = ((l * 2 + h) * 9) * 128
            ps = [pspool.tile([128, TILE_PX], f32, tag=f"ps{j // 4}{j % 4}",
                              name=f"g8{h}{j}") for j in range(8)]
            for tap in range(9):
                dh, dw = tap // 3, tap % 3
                wc = wbase + tap * 128
                D0 = wg_sb[0:64, wc:wc + 64]
                D1 = wg_sb[64:128, wc:wc + 64]
                st, sp = (tap == 0), (tap == 8)

                def rhs(r, band):
                    r0 = r * ROWS_PER_TILE
                    return src[64 * band:64 * band + 64,
                               r0 + dh:r0 + dh + ROWS_PER_TILE, dw:dw + W]

                for r in (0, 2, 4, 6):  # T0: natural lo
                    nc.tensor.matmul(ps[r][0:64, :], lhsT=D0, rhs=rhs(r, 0),
                                     start=st, stop=sp)
                for r in (0, 2, 4, 6):  # T10: natural hi
                    nc.tensor.matmul(ps[r][64:128, :], lhsT=D1, rhs=rhs(r, 1),
                                     start=st, stop=sp)
                for r in (1, 3, 5, 7):  # T2: rotated lo
                    nc.tensor.matmul(ps[r][64:128, :], lhsT=D0, rhs=rhs(r, 0),
                                     start=st, stop=sp)
                for r in (1, 3, 5, 7):  # T8: rotated hi
                    nc.tensor.matmul(ps[r][0:64, :], lhsT=D1, rhs=rhs(r, 1),
                                     start=st, stop=sp)
            for r in range(8):
                relu_pass(td_dst(h, r), ps[r][:], _bias_col(2 * l, h, _rho(r)),
                          use_act=(r % 2 == 0))

    def gconv_stage_paired(src_pads, l):
        # Region-PAIR matmuls: one MM streams regions (r, r+2) through a
        # gapped rhs (stride 16 rows) into a 2-bank PSUM tile (regions at
        # f32 offsets 0 and 512).  Emitted as a legal single-region MM on
        # the pair's LOWER region; widen_pair_matmuls() rewrites the APs
        # post-schedule.  Only safe when the source tensor is written
        # wholesale (xpad): the pre-surgery dependency AP does not cover
        # the second region's rows.
        for s, h in ((0, 0), (0, 1), (1, 0), (1, 1)):
            src = p3(src_pads[h])
            wbase = ((l * 2 + h) * 9) * 128
            groups = [(0, 2), (1, 3)] if s == 0 else [(4, 6), (5,)]
            pt = [pspool.tile([128, 1024], f32, tag=f"pp{h}{k}",
                              name=("pp" if len(g) == 2 else "pq")
                              + f"{l}{s}{h}{k}")
                  for k, g in enumerate(groups)]
            for tap in range(9):
                dh, dw = tap // 3, tap % 3
                wc = wbase + tap * 128
                D0a = wg_sb[0:64, wc:wc + 64]          # T0  (0, 0)
                D0b = wg_sb[0:64, wc + 64:wc + 128]    # T2  (0, 64)
                D1a = wg_sb[64:128, wc:wc + 64]        # T8  (64, 0)
                D1b = wg_sb[64:128, wc + 64:wc + 128]  # T10 (64, 64)
                st, sp = (tap == 0), (tap == 8)

                def rhs(r, band):
                    r0 = r * ROWS_PER_TILE
                    return src[64 * band:64 * band + 64,
                               r0 + dh:r0 + dh + ROWS_PER_TILE, dw:dw + W]

                for k, g in enumerate(groups):
                    ra = g[0]
                    if ra % 2 == 0:  # natural
                        nc.tensor.matmul(pt[k][0:64, 0:TILE_PX], lhsT=D0a,
                                         rhs=rhs(ra, 0), start=st, stop=sp)
                        nc.tensor.matmul(pt[k][64:128, 0:TILE_PX], lhsT=D1b,
                                         rhs=rhs(ra, 1), start=st, stop=sp)
                    else:  # rotated
                        nc.tensor.matmul(pt[k][64:128, 0:TILE_PX], lhsT=D0b,
                                         rhs=rhs(ra, 0), start=st, stop=sp)
                        nc.tensor.matmul(pt[k][0:64, 0:TILE_PX], lhsT=D1a,
                                         rhs=rhs(ra, 1), start=st, stop=sp)
            for k, g in enumerate(groups):
                for gi, r in enumerate(g):
                    relu_pass(td_dst(h, r), pt[k][:, 512 * gi:512 * gi + TILE_PX],
                              _bias_col(2 * l, h, _rho(r)),
                              use_act=((k + gi) % 2 == 0))

    def gconv_stage(src_pads, l, pairable=False):
        if _G32[0]:
            gconv_stage_g32(src_pads, l)
            return
        if _TAP8[0]:
            gconv_stage_tap8(src_pads, l)
            return
        if pairable and _PAIR[0]:
            gconv_stage_paired(src_pads, l)
            return
        # tap-outer sets of 4 regions (4 PSUM banks each); halves
        # interleaved (h0s0, h1s0, h0s1, h1s1) so both halves' early
        # regions are relu'd before the stage ends and pw can start sooner
        for s, h in ((0, 0), (0, 1), (1, 0), (1, 1)):
            src = p3(src_pads[h])
            wbase = ((l * 2 + h) * 9) * 128
            if True:
                regs = [0, 1, 2, 3] if s == 0 else [4, 5, 6]
                pt = [pspool.tile([128, 1024], f32, tag=f"pp{h}{k}",
                                  name=f"g2{l}{s}{h}{k}") for k in range(2)]
                ps = [pt[j % 2][:, (j // 2) * 512:(j // 2) * 512 + TILE_PX]
                      for j in range(len(regs))]
                for tap in range(9):
                    dh, dw = tap // 3, tap % 3
                    wc = wbase + tap * 128
                    # copies of D0/D1 chosen per col-rect so the widen
                    # pass can fuse each row half's two 64-col LDW into
                    # one 128-col (FWL) load
                    D0a = wg_sb[0:64, wc:wc + 64]          # T0  (0, 0)
                    D0b = wg_sb[0:64, wc + 64:wc + 128]    # T2  (0, 64)
                    D1a = wg_sb[64:128, wc:wc + 64]        # T8  (64, 0)
                    D1b = wg_sb[64:128, wc + 64:wc + 128]  # T10 (64, 64)
                    st, sp = (tap == 0), (tap == 8)

                    def rhs(j, band):
                        r0 = regs[j] * ROWS_PER_TILE
                        return src[64 * band:64 * band + 64,
                                   r0 + dh:r0 + dh + ROWS_PER_TILE, dw:dw + W]

                    # MATMUL starts are pc-monotone (strict FIFO), so
                    # same-tile MMs must be maximally separated: regions
                    # alternate natural (T0/T10) and rotated (T2/T8)
                    # tiles, so ascending j rotates through all four.
                    for j in range(len(regs)):
                        if regs[j] % 2 == 0:  # natural
                            nc.tensor.matmul(ps[j][0:64, :], lhsT=D0a,
                                             rhs=rhs(j, 0), start=st, stop=sp)
                            nc.tensor.matmul(ps[j][64:128, :], lhsT=D1b,
                                             rhs=rhs(j, 1), start=st, stop=sp)
                        else:  # rotated
                            nc.tensor.matmul(ps[j][64:128, :], lhsT=D0b,
                                             rhs=rhs(j, 0), start=st, stop=sp)
                            nc.tensor.matmul(ps[j][0:64, :], lhsT=D1a,
                                             rhs=rhs(j, 1), start=st, stop=sp)
                for j in range(len(regs)):
                    r = regs[j]
                    relu_pass(td_dst(h, r), ps[j][:], _bias_col(2 * l, h, _rho(r)),
                              use_act=(j % 2 == 0))

    def pw_stage(dst_fn, l):
        for Hh in range(2):
            for r in range(NREG):
                pt = pspool.tile([128, 1024], f32, tag=f"pp{Hh}{r % 2}",
                                 name=f"pw{l}{Hh}{r}")
                off = ((r // 2) % 2) * 512
                ps = pt[:, off:off + TILE_PX]
                for A in range(2):
                    idx = ((((l * 2 + Hh) * 2 + A) * nrho)
                           + _RHOS().index(_rho(r))) * 128
                    nc.tensor.matmul(ps, lhsT=wp_sb[:, idx:idx + 128],
                                     rhs=td[A][:, r * TILE_PX:(r + 1) * TILE_PX],
                                     start=(A == 0), stop=(A == 1))
                relu_pass(dst_fn(Hh, r), ps, _bias_col(2 * l + 1, Hh, _rho(r)),
                          use_act=((Hh + r) % 2 == 0))

    def load_dma(n):
        for h in range(2):
            nc.sync.dma_start(xstage[h][:], x_ap[n, 128 * h:128 * (h + 1), :])

    def load_copy(n):
        # emitted well after the dma_start so the ACT/DVE copy doesn't sit
        # at the head of the engine queue stalling on the DMA with relu
        # work piled behind it
        for h in range(2):
            dst = p3(xpad[n % 2][h])[:, 1:57, 1:57]
            src = xstage[h][:].rearrange("p (a b) -> p a b", b=W)
            if h == 0:
                nc.scalar.copy(dst, src)
            else:
                nc.vector.tensor_copy(dst, src)

    def load_sample(n):
        load_dma(n)
        load_copy(n)

    def one_batch(tail_prefetch):
      for n in range(BPC):
        nload = None
        if not _SKIP["dma"]:
            if n + 1 < BPC:
                nload = n + 1
            elif tail_prefetch:
                nload = 0  # next iteration's sample 0 (idempotent reload)
            if nload is not None:
                load_dma(nload)

        ysn = ysb[n % NB]

        if not _SKIP["gconv"]:
            gconv_stage(xpad[n % 2], 0, pairable=True)

        if nload is not None:
            load_copy(nload)

        def r2_dst(Hh, r):
            return p3(r2pad[Hh])[:, r * ROWS_PER_TILE + 1:
                                 r * ROWS_PER_TILE + 1 + ROWS_PER_TILE, 1:57]
        if not _SKIP["pw"]:
            pw_stage(r2_dst, 0)

        if not _SKIP["gconv"]:
            gconv_stage(r2pad, 1)

        def y_dst(Hh, r):
            return ysn[Hh][:, r * TILE_PX:(r + 1) * TILE_PX]
        if not _SKIP["pw"]:
            pw_stage(y_dst, 1)

        # store in region-pair chunks so the DMA overlaps the remaining
        # evacuations instead of bursting at the sample tail
        if not _SKIP["dma"]:
            for h in range(2):
                for cstart in range(0, NPIX, 2 * TILE_PX):
                    cend = min(cstart + 2 * TILE_PX, NPIX)
                    nc.sync.dma_start(y_ap[n, 128 * h:128 * (h + 1), cstart:cend],
                                      ysn[h][:, cstart:cend])

    if any(_SKIP.values()):
        # ablation builds: pre-write every tile a skipped stage would have
        # produced, so Tile's read-before-write check passes
        for tile_ in ([xstage[h] for h in range(2)] + xpad[0] + xpad[1] +
                      r2pad + td + ysb[0] + ysb[1]):
            nc.gpsimd.memset(tile_[:], 0.0)

    if not _SKIP["dma"]:
        load_sample(0)
    if _HWLOOP[0] > 1:
        with tc.For_i(0, _HWLOOP[0]):
            one_batch(True)
    else:
        for rep in range(repeat):
            one_batch(rep + 1 < repeat)


def _rect(inst):
    tp = inst.tile_position or (0, 0)
    ts = inst.tile_size or (128, 128)
    return (tp[0], tp[0] + ts[0], tp[1], tp[1] + ts[1])


def _overlaps(a, b):
    return a[0] < b[1] and b[0] < a[1] and a[2] < b[3] and b[2] < a[3]


def prune_ldweights(nc):
    """Remove LDWEIGHTS that reload what is already in the PE array.

    The tile legalizer pairs every InstMatmult with its own InstLdweights;
    when consecutive (per array-region) loads are identical and carry no
    semaphore role, all but the first are redundant.  This keeps the PE
    weight port from becoming the bottleneck for small-tile matmuls.
    """
    removed = 0
    for bb in nc.m.functions[0].blocks:
        insts = bb.instructions
        keep = []
        live = {}  # tile_position rect -> (key)
        for i in insts:
            if type(i).__name__ == "InstLdweights":
                rect = _rect(i)
                key = (str(i.ins[0]), rect)
                prev = live.get(rect)
                if prev == key and i.sync_info is None:
                    removed += 1
                    continue
                # invalidate overlapping regions, record this load
                for r in [r for r in live if _overlaps(r, rect)]:
                    del live[r]
                live[rect] = key
            keep.append(i)
        if removed:
            bb.instructions = keep
    return removed


def widen_g32_ldweights(nc):
    """Fuse each row band's four 32-col G32 gconv LDWEIGHTS into one
    128-col load of the 4x-tiled wide block (FWL-eligible), and drop the
    narrow loads it covers.

    Emission guarantees LDW (q, c) reads cols base+32c of wg_sb where all
    four 32-col copies hold the band's diag block, so after the first LDW
    of a (band, tap) group is widened to [base, base+128) at
    tile_position (32q, 0), the later narrow loads of the same group are
    redundant (their rect already holds the right content).
    """
    widened = removed = 0
    for bb in nc.m.functions[0].blocks:
        insts = bb.instructions
        keep = []
        live = {}  # row band q -> base col of the wide block loaded
        for i in insts:
            if type(i).__name__ == "InstLdweights":
                ap = i.ins[0]
                bw = i.tile_size[0] if i.tile_size else 0
                if (ap.memref.startswith("wg_sb")
                        and i.tile_size == (32, 32)):
                    q, cc = i.tile_position
                    base = ap.offset - cc
                    if live.get(q) == base:
                        if i.sync_info is None:
                            removed += 1
                            continue
                        # carries a semaphore role: keep as narrow reload
                        # (content is correct either way)
                    else:
                        pairs = list(ap.ap)
                        assert pairs[-1] == (1, bw), pairs
                        import bass_rust
                        ap.offset = base
                        ap.ap = bass_rust.VecI64Pair(
                            [list(p) for p in pairs[:-1]] + [[1, 128]])
                        i.tile_size = (bw, 128)
                        i.tile_position = (q, 0)
                        live[q] = base
                        widened += 1
                else:
                    # any other weight load may clobber the array rows
                    live.clear()
            keep.append(i)
        if removed:
            bb.instructions = keep
    return widened, removed


def widen_pair_matmuls(nc):
    """Rewrite region-pair gconv matmuls (PSUM memref "pp*") to cover both
    regions: ifmap gains a (16*WP, 2) region dim, the PSUM out gains a
    (512, 2) bank dim.  Emission wrote the pair's lower region only."""
    import bass_rust
    widened = 0
    for bb in nc.m.functions[0].blocks:
        for i in bb.instructions:
            if type(i).__name__ != "InstMatmult":
                continue
            out = i.outs[0]
            if not out.memref.startswith("pp"):
                continue
            ifm = i.ins[0]
            ip = [list(p) for p in ifm.ap]
            op = [list(p) for p in out.ap]
            assert ip[-2:] == [[WP, ROWS_PER_TILE], [1, W]], (ip, i.name)
            assert op[-1] == [1, TILE_PX], (op, i.name)
            ifm.ap = bass_rust.VecI64Pair(
                ip[:-2] + [[16 * WP, 2]] + ip[-2:])
            out.ap = bass_rust.VecI64Pair(op[:-1] + [[512, 2]] + op[-1:])
            widened += 1
    return widened


def build_program(repeat=1):
    import contextlib

    import concourse.tile as tile
    from concourse import bacc, mybir

    f32 = mybir.dt.float32
    f16 = mybir.dt.float16
    nc = bacc.Bacc("TRN2", target_bir_lowering=False, debug=False,
                   num_devices=N_CORES)
    x_d = nc.dram_tensor("x", [BPC, C, NPIX], f16, kind="ExternalInput").ap()
    wg_d = nc.dram_tensor("wg", [128, 2 * 2 * 9 * 128], f16,
                          kind="ExternalInput").ap()
    wp_d = nc.dram_tensor("wp", [128, 2 * 2 * 2 * len(_RHOS()) * 128], f16,
                          kind="ExternalInput").ap()
    bias_d = nc.dram_tensor("bias", [128, 24], f32, kind="ExternalInput").ap()
    zeros_d = nc.dram_tensor("zeros", [128, 64], f16, kind="ExternalInput").ap()
    y_d = nc.dram_tensor("y", [BPC, C, NPIX], f16, kind="ExternalOutput").ap()

    with tile.TileContext(nc) as tc:
        with contextlib.ExitStack() as ctx:
            tc._build_ctx = ctx
            _build_body(tc, y_d, x_d, wg_d, wp_d, bias_d, zeros_d, repeat=repeat)
    widen_g32_ldweights(nc)
    if _PAIR[0]:
        widen_pair_matmuls(nc)
    prune_ldweights(nc)
    nc.compile()
    return nc


def get_program():
    key = ("nc", _REPEAT[0], _HWLOOP[0], _PW8[0], _G32[0], _TAP8[0],
           _PAIR[0], tuple(sorted(_SKIP.items())))
    if key not in _CACHED:
        _CACHED[key] = build_program(repeat=_REPEAT[0])
    return _CACHED[key]


def make_in_maps(inputs):
    wg, wp, bias = prepare_weights(inputs)
    x = np.ascontiguousarray(np.asarray(inputs["x"], np.float16))
    in_maps = []
    for i in range(N_CORES):
        in_maps.append({
            "x": x[i * BPC:(i + 1) * BPC].reshape(BPC, C, NPIX),
            "wg": wg, "wp": wp, "bias": bias,
            "zeros": np.zeros((128, 64), np.float16),
        })
    return in_maps


def _run(inputs, trace=False):
    from concourse.bass_utils import run_bass_kernel_spmd

    nc = get_program()
    in_maps = make_in_maps(inputs)
    res = run_bass_kernel_spmd(nc, in_maps, list(range(N_CORES)), trace=trace)
    out = np.concatenate(
        [res.results[i]["y"].reshape(BPC, C, H, W).astype(np.float32)
         for i in range(N_CORES)],
        axis=0)
    return out, res


def kernel(**inputs):
    return _run(inputs)[0]



# revision 22
# speedup vs baseline: 1.1722x; 1.1722x over previous
"""Trainium2 Bass kernel for the CLC block (grouped 3x3 conv -> BN+ReLU ->
grouped 1x1 conv -> BN+ReLU, twice).

Sharding: pure data parallel, batch 32 -> 4 samples per core on 8 cores.

Per-core design (f16 storage/matmul, f32 PSUM accumulate):
  - HBM I/O is f16: the host pre-converts x (the kernel quantizes to f16
    on-chip anyway) and post-converts y back to f32 (~5e-4 rounding,
    far below the 2e-2 gate).  Halves DMA traffic both directions.
  - Channel-major layout: [128 channel partitions, pixels] per
    128-channel half; 7 pixel regions of 8 rows (448 px = one PSUM bank).
  - gconv3x3 is block-diagonal at 64-channel granularity in the g-major
    output layout (out pos m = 4*g_loc + i).  Each tap is computed with the
    two 64x64 diagonal blocks loaded into all four PE-array quadrants:
      T0 (rows 0:64,  psum 0:64)   D0 -> even regions outputs 0:64
      T10(rows 64:128,psum 64:128) D1 -> even regions outputs 64:128
      T2 (rows 0:64,  psum 64:128) D0 -> odd regions outputs 0:64
      T8 (rows 64:128,psum 0:64)   D1 -> odd regions outputs 64:128
    Odd ("rotated") regions' PSUM banks hold the two 64-channel halves
    swapped -- downstream pw weights compensate, no swap copies needed.
    The four quadrant tiles stream concurrently (~4 distinct rhs streams
    sustained, which saturates the PE array's MAC capacity); emission
    alternates tiles because MATMUL starts are pc-monotone.
  - Tap-outer ordering over sets of regions (PSUM banks) makes
    consecutive matmuls share lhsT; a post-schedule pass prunes the
    redundant LDWEIGHTS the legalizer inserts.
  - pw 1x1 conv: per region, 2 accumulating K=128 full-array matmuls
    (one per g-major input half), lhsT variant chosen by region rotation.
  - BN + conv-bias fold into matmul weights host-side; each stage needs a
    single bias+ReLU pass evacuating PSUM->SBUF (ACT/DVE alternating).
  - Input pad-copies are emitted AFTER the first gconv stage so the
    ACT/DVE queues don't stall on the input DMA with relu work behind
    them; gconv emits halves interleaved (h0s0, h1s0, h0s1, h1s1) so pw
    can start as soon as the PE drains.

Measured on HW (hwloop-contrast wall timing, best of trials): ~160 us
per batch vs 199 us for the session-start baseline.  Ablations: with
relu+DMA removed the floor is ~164 us (pre-balance), i.e. the kernel is
PE-matmul bound; the gconv streams 9 taps x 2 bands over 4 concurrent
64-row streams.  The REGIONS table splits the 7th region into two
4-row halves (one natural, one rotated) so every region set feeds all
four quadrant lanes 2+2 -- an unbalanced 4/3 split left T2/T8 idle
while T0/T10 streamed the odd region (~14% of gconv time).

Dead ends measured this session (do not retry naively):
  - 32x32 tiling (_G32): 16 tiles do NOT stream 16 distinct rhs
    concurrently; ~one stream per 32-col group at best.  576 us.
  - Widening gconv LDWEIGHTS to 128 cols (FWL): a full-width load
    conflicts with every in-flight quadrant stream -> serializes the
    array.  396 us.  Narrow per-quadrant LDWEIGHTS pipeline fine.
  - _TAP8 (all regions in one tap-outer loop): 182 us -- longer relu
    drain tail beats the LDWEIGHTS savings.
  - pw in fp8 DoubleRow (_PW8): ~4e-2 rel err, over the 2e-2 gate
    (prior session).
  - Region-pair matmuls: tried BOTH a gapped 4D ifmap (regions r, r+2
    via AP surgery, _PAIR flag) and a contiguous 16-row window (adjacent
    regions sharing rho) with a 3D strided 2-bank PSUM out.  Walrus
    rejects both ("generates invalid ISA instruction"): the matmul PSUM
    out AP must be contiguous within ONE bank, so N > 512 f32 per MM is
    architecturally impossible.  MM count cannot be reduced below one
    per (region, band).
  - PSUM tile-pool tags allocate STATICALLY (no liveness reuse): all
    stages must share the four 2-bank "pp" tags (16KB = all 8 banks);
    gconv2/pw address regions as 512-f32 offset halves of those tiles.
"""

import numpy as np

B, C, H, W = 32, 256, 56, 56
EPS = 1e-5
N_CORES = 8
BPC = B // N_CORES  # samples per core
HP, WP = H + 2, W + 2  # padded spatial
NPIX = H * W
NPAD = HP * WP
ROWS_PER_TILE = 8
NREG = H // ROWS_PER_TILE  # 7 pixel regions
TILE_PX = ROWS_PER_TILE * W  # 448 (fits a 512-f32 PSUM bank)
# (px_start, npx, rho) per gconv/pw work region.  56 rows give 4 natural +
# 3 rotated 8-row regions -- unbalanced quadrant lanes (T0/T10 get 4 serial
# MM streams per tap, T2/T8 only 3).  Splitting the last region into two
# 4-row halves (one natural, one rotated) balances every set 2+2.
REGIONS = [(0, 448, 0), (448, 448, 2), (896, 448, 0), (1344, 448, 2),
           (1792, 448, 0), (2240, 448, 2), (2688, 224, 0), (2912, 224, 2)]


# ---------------------------------------------------------------------------
# Host-side weight preparation
# ---------------------------------------------------------------------------

def _bn_fold(bg, bb, bm, bv):
    inv = bg / np.sqrt(bv + EPS)
    return inv, bb - bm * inv


def prepare_weights(inp):
    """Returns (wg, wp [128, 2*2*2*4*128], bias [128, 24]), all f16/f32.

    Rotation rho (0..3): a PSUM bank in rotation rho holds natural output
    position p' at partition p = (p' + 32*rho) % 128 -- i.e.
    psum = roll(nat, 32*rho).  The 64x64 scheme uses rho in {0, 2}; the
    32x32 scheme uses rho = region % 4.

    wg (G32 off): [128, ((l*2+h)*9 + t)*64 + m64] diag-block lhsT; rows
      0:64 = D0 (outputs 0:64), rows 64:128 = D1.
    wg (G32 on):  [128, ((l*2+h)*9 + t)*32 + m32]; rows 32q:32q+32 = Dq
      (outputs 32q:32q+32 of the half).
    wp[k, ((((l*2+Hh)*2 + A)*4 + rho)*128 + m]: lhsT for pw layer l, output
      half Hh (natural), g-major input half A, input rotation rho.
    bias cols: gconv = rho*4 + 2*l + h (0..15), pw = 16 + 2*l + Hh.
    """
    f32 = np.float32
    wg_full = np.zeros((128, 2, 2, 9, 128), f32)
    wp_nat = np.zeros((128, 2, 2, 2, 128), f32)
    bias = np.zeros((128, 24), f32)

    gconv_params = [
        (inp["w1"], inp["b1"], inp["bn1a_g"], inp["bn1a_b"], inp["bn1a_m"], inp["bn1a_v"]),
        (inp["w2"], inp["b2"], inp["bn2a_g"], inp["bn2a_b"], inp["bn2a_m"], inp["bn2a_v"]),
    ]
    pw_params = [
        (inp["pw1"], inp["pb1"], inp["bn1b_g"], inp["bn1b_b"], inp["bn1b_m"], inp["bn1b_v"]),
        (inp["pw2"], inp["pb2"], inp["bn2b_g"], inp["bn2b_b"], inp["bn2b_m"], inp["bn2b_v"]),
    ]

    for l, (w, bcv, bg, bb, bm, bv) in enumerate(gconv_params):
        w = np.asarray(w, f32)
        inv, shift = _bn_fold(np.asarray(bg, f32), np.asarray(bb, f32),
                              np.asarray(bm, f32), np.asarray(bv, f32))
        bconv = np.asarray(bcv, f32).reshape(256)  # index i*64+g
        beff = bconv * inv + shift  # natural order o
        for h in range(2):
            bnat = np.zeros(128, f32)
            for m in range(128):
                g = 32 * h + m // 4
                i = m % 4
                o = i * 64 + g
                for kk in range(4):
                    k = 4 * (g - 32 * h) + kk
                    for t in range(9):
                        wg_full[k, l, h, t, m] = w[i, g, kk, t // 3, t % 3] * inv[o]
                bnat[m] = beff[o]
            for rho in range(4):
                bias[:, rho * 4 + 2 * l + h] = np.roll(bnat, 32 * rho)

    # extract diagonal blocks: rows of band b -> output columns of band b
    if _G32[0]:
        # wide layout: per (l, h, tap) a 128-col block holding the band's
        # 32x32 diag block tiled 4x horizontally, so one 128-col LDWEIGHTS
        # (FWL-eligible) loads all four col-rects of the row band
        wg = np.zeros((128, 2, 2, 9, 128), f32)
        for b in range(4):
            Db = wg_full[32 * b:32 * b + 32, :, :, :, 32 * b:32 * b + 32]
            for c in range(4):
                wg[32 * b:32 * b + 32, :, :, :, 32 * c:32 * c + 32] = Db
    else:
        # wide layout: per (l, h, tap) a 128-col block; rows 0:64 hold
        # [D0|D0] (tiles T0 and T2 both want D0), rows 64:128 hold
        # [D1|D1] (T8 and T10), so one 128-col FWL LDWEIGHTS per row
        # half loads both col-rects
        wg = np.zeros((128, 2, 2, 9, 128), f32)
        for b in range(2):
            Db = wg_full[64 * b:64 * b + 64, :, :, :, 64 * b:64 * b + 64]
            for c in range(2):
                wg[64 * b:64 * b + 64, :, :, :, 64 * c:64 * c + 64] = Db

    for l, (w, pb, bg, bb, bm, bv) in enumerate(pw_params):
        w = np.asarray(w, f32).reshape(256, 64)
        inv, shift = _bn_fold(np.asarray(bg, f32), np.asarray(bb, f32),
                              np.asarray(bm, f32), np.asarray(bv, f32))
        beff = np.asarray(pb, f32) * inv + shift
        for Hh in range(2):
            for m in range(128):
                c = 128 * Hh + m
                i = c // 64
                for g in range(64):
                    p = 4 * g + i  # global g-major position of input (i, g)
                    A, k = divmod(p, 128)
                    wp_nat[k, l, Hh, A, m] = w[c, g] * inv[c]
                bias[m, 16 + 2 * l + Hh] = beff[c]

    # input-rotation variants: lhsT_rho[p] = lhsT_nat[(p - 32*rho) % 128].
    # Only the variants a scheme uses are materialized (64x64: rho 0 and 2).
    rhos = _RHOS()
    wp = np.stack([np.roll(wp_nat, 32 * rho, axis=0) for rho in rhos],
                  axis=4)  # [128, l, Hh, A, rho-slot, 128]
    return (wg.reshape(128, -1).astype(np.float16),
            wp.reshape(128, 2 * 2 * 2 * len(rhos) * 128).astype(np.float16),
            bias)


def _RHOS():
    return (0, 1, 2, 3) if _G32[0] else (0, 2)


def _rho(r):
    return (r % 4) if _G32[0] else 2 * (r % 2)


def _bias_col(stage, h, rho):
    # stage 0..3 = gconv1, pw1, gconv2, pw2; l = stage // 2
    if stage % 2 == 0:  # gconv
        return rho * 4 + 2 * (stage // 2) + h
    return 16 + 2 * (stage // 2) + h


# ---------------------------------------------------------------------------
# Numpy emulation of the exact kernel dataflow (for validation)
# ---------------------------------------------------------------------------

def emulate(inp):
    wg, wp, bias = prepare_weights(inp)
    bw = 32 if _G32[0] else 64
    wg = wg.astype(np.float32).reshape(128, 2, 2, 9, -1)
    # wide tiled layouts: every bw-col copy is identical; the diag block
    # for band b is any copy of rows b*bw:(b+1)*bw
    wg = wg[:, :, :, :, 0:bw]
    wp = wp.astype(np.float32).reshape(128, 2, 2, 2, len(_RHOS()), 128)
    x = np.asarray(inp["x"], np.float32)
    out = np.zeros_like(x)

    for n in range(B):
        xpad = np.zeros((2, 128, HP, WP), np.float32)
        for h in range(2):
            xpad[h, :, 1:57, 1:57] = x[n, 128 * h:128 * (h + 1)].astype(np.float16)

        def gconv(src_pad, l):
            td = [np.zeros((128, H, W), np.float32) for _ in range(2)]
            for h in range(2):
                for pxs, npx, rho in REGIONS:
                    r0, nrows = pxs // W, npx // W
                    acc = np.zeros((128, nrows, W), np.float32)
                    for tap in range(9):
                        dh, dw = tap // 3, tap % 3
                        rhs = src_pad[h][:, r0 + dh:r0 + dh + nrows,
                                         dw:dw + W].reshape(128, -1)
                        nat = np.concatenate([
                            wg[b * bw:(b + 1) * bw, l, h, tap, :].T
                            @ rhs[b * bw:(b + 1) * bw]
                            for b in range(128 // bw)], 0)
                        acc += np.roll(nat, 32 * rho, axis=0).reshape(
                            128, nrows, W)
                    bcol = _bias_col(2 * l, h, rho)
                    res = np.maximum(acc + bias[:, bcol][:, None, None], 0.0)
                    td[h][:, r0:r0 + nrows] = res.astype(np.float16)
            return td

        def pw(td, l):
            dst = [None, None]
            for Hh in range(2):
                o = np.zeros((128, H, W), np.float32)
                for pxs, npx, rho in REGIONS:
                    r0, nrows = pxs // W, npx // W
                    acc = np.zeros((128, npx), np.float32)
                    for A in range(2):
                        rhs = td[A][:, r0:r0 + nrows].reshape(128, -1)
                        acc += wp[:, l, Hh, A, _RHOS().index(rho), :].T @ rhs
                    bcol = _bias_col(2 * l + 1, Hh, rho)
                    res = np.maximum(acc + bias[:, bcol][:, None], 0.0)
                    o[:, r0:r0 + nrows] = res.reshape(128, nrows, W)
                dst[Hh] = o
            return dst

        t1 = gconv(xpad, 0)
        t2 = pw(t1, 0)
        t2pad = np.zeros((2, 128, HP, WP), np.float32)
        for h in range(2):
            t2pad[h, :, 1:57, 1:57] = t2[h].astype(np.float16)
        t3 = gconv(t2pad, 1)
        y = pw(t3, 1)
        out[n, 0:128] = y[0]
        out[n, 128:256] = y[1]
    return out


# ---------------------------------------------------------------------------
# Bass program
# ---------------------------------------------------------------------------

_CACHED = {}
_REPEAT = [1]
_HWLOOP = [1]
_PW8 = [False]  # pw in fp8 DoubleRow: fast but ~4e-2 rel err -- too lossy
# 32x32 PE tiling measured 249 us/iter vs 173 us for 64x64 on HW (the
# 16 serialized 32-col LDWEIGHTS per tap dominate) -- keep 64x64.
_G32 = [False]
# UNTESTED candidate (census-driven, see memory): single 8-region tap-outer
# emission so one LDWEIGHTS covers both region-sets' matmuls per quadrant
# (-576 LDW/program ~ -30us weight path, vs ~2us/sample extra PSUM boundary
# stalls from holding all 8 banks).  Flip with set_tap8(True) and verify.
_TAP8 = [False]
_PAIR = [False]  # region-pair MMs via AP surgery: walrus rejects the
# 4D gapped ifmap ("invalid ISA instruction") -- PE ifmap APs are 3D max


_SKIP = {"gconv": False, "pw": False, "dma": False, "relu": False}


def set_skip(which, v=True):
    _SKIP[which] = bool(v)


def set_tap8(v):
    _TAP8[0] = bool(v)


def set_pair(v):
    _PAIR[0] = bool(v)


def set_pw8(v):
    _PW8[0] = bool(v)


def set_g32(v):
    _G32[0] = bool(v)


def set_repeat(r):
    _REPEAT[0] = r


def set_hwloop(r):
    _HWLOOP[0] = r


def _build_body(tc, y_ap, x_ap, wg_ap, wp_ap, bias_ap, zeros_ap, repeat=1):
    import concourse.bass as bass  # noqa: F401
    from concourse import mybir

    nc = tc.nc
    f32 = mybir.dt.float32
    f16 = mybir.dt.float16
    f8 = mybir.dt.float8e4
    ADD = mybir.AluOpType.add
    MAX = mybir.AluOpType.max
    RELU = mybir.ActivationFunctionType.Relu
    DR = mybir.MatmulPerfMode.DoubleRow

    ctx = tc._build_ctx

    const = ctx.enter_context(tc.tile_pool(name="const", bufs=1))
    persist = ctx.enter_context(tc.tile_pool(name="persist", bufs=1))
    pspool = ctx.enter_context(tc.tile_pool(name="ps", bufs=1, space="PSUM"))

    bw = 32 if _G32[0] else 64
    wg_sb = const.tile([128, 2 * 2 * 9 * 128], f16, tag="wg", name="wg_sb")
    nrho = len(_RHOS())
    wp_sb = const.tile([128, 2 * 2 * 2 * nrho * 128], f16, tag="wp", name="wp_sb")
    bias_sb = const.tile([128, 24], f32, tag="bias", name="bias_sb")
    nc.sync.dma_start(wg_sb[:], wg_ap)
    nc.sync.dma_start(wp_sb[:], wp_ap)
    nc.sync.dma_start(bias_sb[:], bias_ap)

    # double-buffered padded input: sample n+1's load+pad-copy runs during
    # sample n's compute so the PE never waits on the ACT/DVE copy
    xpad = [[persist.tile([128, NPAD], f16, tag=f"xpad{b}{h}", name=f"xpad{b}{h}")
             for h in range(2)] for b in range(2)]
    xstage = [persist.tile([128, NPIX], f16, tag=f"xstage{h}", name=f"xstage{h}") for h in range(2)]
    r2pad = [persist.tile([128, NPAD], f16, tag=f"r2pad{h}", name=f"r2pad{h}") for h in range(2)]
    td = [persist.tile([128, NPIX], f16, tag=f"td{h}", name=f"td{h}") for h in range(2)]
    td_dst = lambda h, r: td[h][:, r * TILE_PX:(r + 1) * TILE_PX]
    NB = 2
    ysb = [[persist.tile([128, NPIX], f16, tag=f"ysb{b}{h}", name=f"ysb{b}{h}") for h in range(2)] for b in range(NB)]

    def p3(tile_):
        return tile_[:].rearrange("p (a b) -> p a b", b=WP)

    for t in xpad[0] + xpad[1] + r2pad:
        v = p3(t)
        flat = t[:]
        nc.sync.dma_start(flat[:, 0:WP], zeros_ap[:, 0:WP])
        nc.sync.dma_start(flat[:, (HP - 1) * WP:HP * WP], zeros_ap[:, 0:WP])
        nc.sync.dma_start(v[:, 1:HP - 1, 0:1], zeros_ap[:, 0:HP - 2])
        nc.sync.dma_start(v[:, 1:HP - 1, WP - 1:WP], zeros_ap[:, 0:HP - 2])

    def relu_pass(dst, ps, scol, use_act):
        if _SKIP["relu"]:
            return
        if use_act:
            nc.scalar.activation(dst, ps, RELU, bias=bias_sb[:, scol:scol + 1])
        else:
            nc.vector.tensor_scalar(dst, ps, bias_sb[:, scol:scol + 1], 0.0,
                                    op0=ADD, op1=MAX)

    def gconv_stage_g32(src_pads, l):
        # 16 concurrent 32x32 tiles; tap-outer over all 8 regions (8 banks).
        # lhsT for (q, c) is copy c of the 4x-tiled wide block, so the
        # widen_g32_ldweights pass can fuse each band's four 32-col LDW
        # into one 128-col (FWL) load.
        for h in range(2):
            src = p3(src_pads[h])
            wbase = ((l * 2 + h) * 9) * 128
            ps = [pspool.tile([128, TILE_PX], f32, tag=f"ps{j // 4}{j % 4}",
                              name=f"g{l}{h}{j}") for j in range(8)]
            for tap in range(9):
                dh, dw = tap // 3, tap % 3
                wc = wbase + tap * 128
                st, sp = (tap == 0), (tap == 8)
                for q in range(4):
                    for c in range(4):
                        rho = (c - q) % 4
                        Dqc = wg_sb[32 * q:32 * q + 32,
                                    wc + 32 * c:wc + 32 * c + 32]
                        for P in range(2):
                            r0 = (P * 4 + rho) * ROWS_PER_TILE
                            nc.tensor.matmul(
                                ps[P * 4 + rho][32 * c:32 * c + 32, :],
                                lhsT=Dqc,
                                rhs=src[32 * q:32 * q + 32,
                                        r0 + dh:r0 + dh + ROWS_PER_TILE,
                                        dw:dw + W],
                                start=st, stop=sp,
                                tile_position=(32 * q, 32 * c))
            for r in range(8):
                relu_pass(td_dst(h, r), ps[r][:], _bias_col(2 * l, h, _rho(r)),
                          use_act=(r % 2 == 0))

    def gconv_stage_tap8(src_pads, l):
        # one tap-outer loop over all 8 regions: both sets' same-quadrant
        # matmuls are adjacent, so one LDWEIGHTS serves 4 MMs per tap
        for h in range(2):
            src = p3(src_pads[h])
            wbase = ((l * 2 + h) * 9) * 128
            ps = [pspool.tile([128, TILE_PX], f32, tag=f"ps{j // 4}{j % 4}",
                              name=f"g8{h}{j}") for j in range(8)]
            for tap in range(9):
                dh, dw = tap // 3, tap % 3
                wc = wbase + tap * 128
                D0 = wg_sb[0:64, wc:wc + 64]
                D1 = wg_sb[64:128, wc:wc + 64]
                st, sp = (tap == 0), (tap == 8)

                def rhs(r, band):
                    r0 = r * ROWS_PER_TILE
                    return src[64 * band:64 * band + 64,
                               r0 + dh:r0 + dh + ROWS_PER_TILE, dw:dw + W]

                for r in (0, 2, 4, 6):  # T0: natural lo
                    nc.tensor.matmul(ps[r][0:64, :], lhsT=D0, rhs=rhs(r, 0),
                                     start=st, stop=sp)
                for r in (0, 2, 4, 6):  # T10: natural hi
                    nc.tensor.matmul(ps[r][64:128, :], lhsT=D1, rhs=rhs(r, 1),
                                     start=st, stop=sp)
                for r in (1, 3, 5, 7):  # T2: rotated lo
                    nc.tensor.matmul(ps[r][64:128, :], lhsT=D0, rhs=rhs(r, 0),
                                     start=st, stop=sp)
                for r in (1, 3, 5, 7):  # T8: rotated hi
                    nc.tensor.matmul(ps[r][0:64, :], lhsT=D1, rhs=rhs(r, 1),
                                     start=st, stop=sp)
            for r in range(8):
                relu_pass(td_dst(h, r), ps[r][:], _bias_col(2 * l, h, _rho(r)),
                          use_act=(r % 2 == 0))

    def gconv_stage_paired(src_pads, l):
        # Region-PAIR matmuls: one MM streams regions (r, r+2) through a
        # gapped rhs (stride 16 rows) into a 2-bank PSUM tile (regions at
        # f32 offsets 0 and 512).  Emitted as a legal single-region MM on
        # the pair's LOWER region; widen_pair_matmuls() rewrites the APs
        # post-schedule.  Only safe when the source tensor is written
        # wholesale (xpad): the pre-surgery dependency AP does not cover
        # the second region's rows.
        for s, h in ((0, 0), (0, 1), (1, 0), (1, 1)):
            src = p3(src_pads[h])
            wbase = ((l * 2 + h) * 9) * 128
            groups = [(0, 2), (1, 3)] if s == 0 else [(4, 6), (5,)]
            pt = [pspool.tile([128, 1024], f32, tag=f"pp{h}{k}",
                              name=("pp" if len(g) == 2 else "pq")
                              + f"{l}{s}{h}{k}")
                  for k, g in enumerate(groups)]
            for tap in range(9):
                dh, dw = tap // 3, tap % 3
                wc = wbase + tap * 128
                D0a = wg_sb[0:64, wc:wc + 64]          # T0  (0, 0)
                D0b = wg_sb[0:64, wc + 64:wc + 128]    # T2  (0, 64)
                D1a = wg_sb[64:128, wc:wc + 64]        # T8  (64, 0)
                D1b = wg_sb[64:128, wc + 64:wc + 128]  # T10 (64, 64)
                st, sp = (tap == 0), (tap == 8)

                def rhs(r, band):
                    r0 = r * ROWS_PER_TILE
                    return src[64 * band:64 * band + 64,
                               r0 + dh:r0 + dh + ROWS_PER_TILE, dw:dw + W]

                for k, g in enumerate(groups):
                    ra = g[0]
                    if ra % 2 == 0:  # natural
                        nc.tensor.matmul(pt[k][0:64, 0:TILE_PX], lhsT=D0a,
                                         rhs=rhs(ra, 0), start=st, stop=sp)
                        nc.tensor.matmul(pt[k][64:128, 0:TILE_PX], lhsT=D1b,
                                         rhs=rhs(ra, 1), start=st, stop=sp)
                    else:  # rotated
                        nc.tensor.matmul(pt[k][64:128, 0:TILE_PX], lhsT=D0b,
                                         rhs=rhs(ra, 0), start=st, stop=sp)
                        nc.tensor.matmul(pt[k][0:64, 0:TILE_PX], lhsT=D1a,
                                         rhs=rhs(ra, 1), start=st, stop=sp)
            for k, g in enumerate(groups):
                for gi, r in enumerate(g):
                    relu_pass(td_dst(h, r), pt[k][:, 512 * gi:512 * gi + TILE_PX],
                              _bias_col(2 * l, h, _rho(r)),
                              use_act=((k + gi) % 2 == 0))

    def gconv_stage(src_pads, l, pairable=False):
        if _G32[0]:
            gconv_stage_g32(src_pads, l)
            return
        if _TAP8[0]:
            gconv_stage_tap8(src_pads, l)
            return
        if pairable and _PAIR[0]:
            gconv_stage_paired(src_pads, l)
            return
        # tap-outer sets of 4 regions (4 PSUM banks each); halves
        # interleaved (h0s0, h1s0, h0s1, h1s1) so both halves' early
        # regions are relu'd before the stage ends and pw can start sooner
        for s, h in ((0, 0), (0, 1), (1, 0), (1, 1)):
            src = p3(src_pads[h])
            wbase = ((l * 2 + h) * 9) * 128
            if True:
                regs = REGIONS[0:4] if s == 0 else REGIONS[4:8]
                pt = [pspool.tile([128, 1024], f32, tag=f"pp{h}{k}",
                                  name=f"g2{l}{s}{h}{k}") for k in range(2)]
                ps = [pt[j % 2][:, (j // 2) * 512:(j // 2) * 512 + regs[j][1]]
                      for j in range(len(regs))]
                for tap in range(9):
                    dh, dw = tap // 3, tap % 3
                    wc = wbase + tap * 128
                    # copies of D0/D1 chosen per col-rect so the widen
                    # pass can fuse each row half's two 64-col LDW into
                    # one 128-col (FWL) load
                    D0a = wg_sb[0:64, wc:wc + 64]          # T0  (0, 0)
                    D0b = wg_sb[0:64, wc + 64:wc + 128]    # T2  (0, 64)
                    D1a = wg_sb[64:128, wc:wc + 64]        # T8  (64, 0)
                    D1b = wg_sb[64:128, wc + 64:wc + 128]  # T10 (64, 64)
                    st, sp = (tap == 0), (tap == 8)

                    def rhs(j, band):
                        pxs, npx, _ = regs[j]
                        r0 = pxs // W
                        return src[64 * band:64 * band + 64,
                                   r0 + dh:r0 + dh + npx // W, dw:dw + W]

                    # MATMUL starts are pc-monotone (strict FIFO), so
                    # same-tile MMs must be maximally separated: regions
                    # alternate natural (T0/T10) and rotated (T2/T8)
                    # tiles, so ascending j rotates through all four.
                    for j in range(len(regs)):
                        if regs[j][2] == 0:  # natural
                            nc.tensor.matmul(ps[j][0:64, :], lhsT=D0a,
                                             rhs=rhs(j, 0), start=st, stop=sp)
                            nc.tensor.matmul(ps[j][64:128, :], lhsT=D1b,
                                             rhs=rhs(j, 1), start=st, stop=sp)
                        else:  # rotated
                            nc.tensor.matmul(ps[j][64:128, :], lhsT=D0b,
                                             rhs=rhs(j, 0), start=st, stop=sp)
                            nc.tensor.matmul(ps[j][0:64, :], lhsT=D1a,
                                             rhs=rhs(j, 1), start=st, stop=sp)
                for j in range(len(regs)):
                    pxs, npx, rho = regs[j]
                    relu_pass(td[h][:, pxs:pxs + npx], ps[j][:],
                              _bias_col(2 * l, h, rho),
                              use_act=(j % 2 == 0))

    def pw_stage(dst_fn, l):
        for Hh in range(2):
            for j, (pxs, npx, rho) in enumerate(REGIONS):
                pt = pspool.tile([128, 1024], f32, tag=f"pp{Hh}{j % 2}",
                                 name=f"pw{l}{Hh}{j}")
                off = ((j // 2) % 2) * 512
                ps = pt[:, off:off + npx]
                for A in range(2):
                    idx = ((((l * 2 + Hh) * 2 + A) * nrho)
                           + _RHOS().index(rho)) * 128
                    nc.tensor.matmul(ps, lhsT=wp_sb[:, idx:idx + 128],
                                     rhs=td[A][:, pxs:pxs + npx],
                                     start=(A == 0), stop=(A == 1))
                relu_pass(dst_fn(Hh, pxs, npx), ps,
                          _bias_col(2 * l + 1, Hh, rho),
                          use_act=((Hh + j) % 2 == 0))

    def load_dma(n):
        for h in range(2):
            nc.sync.dma_start(xstage[h][:], x_ap[n, 128 * h:128 * (h + 1), :])

    def load_copy(n):
        # emitted well after the dma_start so the ACT/DVE copy doesn't sit
        # at the head of the engine queue stalling on the DMA with relu
        # work piled behind it
        for h in range(2):
            dst = p3(xpad[n % 2][h])[:, 1:57, 1:57]
            src = xstage[h][:].rearrange("p (a b) -> p a b", b=W)
            if h == 0:
                nc.scalar.copy(dst, src)
            else:
                nc.vector.tensor_copy(dst, src)

    def load_sample(n):
        load_dma(n)
        load_copy(n)

    def one_batch(tail_prefetch):
      for n in range(BPC):
        nload = None
        if not _SKIP["dma"]:
            if n + 1 < BPC:
                nload = n + 1
            elif tail_prefetch:
                nload = 0  # next iteration's sample 0 (idempotent reload)
            if nload is not None:
                load_dma(nload)

        ysn = ysb[n % NB]

        if not _SKIP["gconv"]:
            gconv_stage(xpad[n % 2], 0, pairable=True)

        if nload is not None:
            load_copy(nload)

        def r2_dst(Hh, pxs, npx):
            r0 = pxs // W
            return p3(r2pad[Hh])[:, r0 + 1:r0 + 1 + npx // W, 1:57]
        if not _SKIP["pw"]:
            pw_stage(r2_dst, 0)

        if not _SKIP["gconv"]:
            gconv_stage(r2pad, 1)

        def y_dst(Hh, pxs, npx):
            return ysn[Hh][:, pxs:pxs + npx]
        if not _SKIP["pw"]:
            pw_stage(y_dst, 1)

        # store in region-pair chunks so the DMA overlaps the remaining
        # evacuations instead of bursting at the sample tail
        if not _SKIP["dma"]:
            for h in range(2):
                for cstart in range(0, NPIX, 2 * TILE_PX):
                    cend = min(cstart + 2 * TILE_PX, NPIX)
                    nc.sync.dma_start(y_ap[n, 128 * h:128 * (h + 1), cstart:cend],
                                      ysn[h][:, cstart:cend])

    if any(_SKIP.values()):
        # ablation builds: pre-write every tile a skipped stage would have
        # produced, so Tile's read-before-write check passes
        for tile_ in ([xstage[h] for h in range(2)] + xpad[0] + xpad[1] +
                      r2pad + td + ysb[0] + ysb[1]):
            nc.gpsimd.memset(tile_[:], 0.0)

    if not _SKIP["dma"]:
        load_sample(0)
    if _HWLOOP[0] > 1:
        with tc.For_i(0, _HWLOOP[0]):
            one_batch(True)
    else:
        for rep in range(repeat):
            one_batch(rep + 1 < repeat)


def _rect(inst):
    tp = inst.tile_position or (0, 0)
    ts = inst.tile_size or (128, 128)
    return (tp[0], tp[0] + ts[0], tp[1], tp[1] + ts[1])


def _overlaps(a, b):
    return a[0] < b[1] and b[0] < a[1] and a[2] < b[3] and b[2] < a[3]


def prune_ldweights(nc):
    """Remove LDWEIGHTS that reload what is already in the PE array.

    The tile legalizer pairs every InstMatmult with its own InstLdweights;
    when consecutive (per array-region) loads are identical and carry no
    semaphore role, all but the first are redundant.  This keeps the PE
    weight port from becoming the bottleneck for small-tile matmuls.
    """
    removed = 0
    for bb in nc.m.functions[0].blocks:
        insts = bb.instructions
        keep = []
        live = {}  # tile_position rect -> (key)
        for i in insts:
            if type(i).__name__ == "InstLdweights":
                rect = _rect(i)
                key = (str(i.ins[0]), rect)
                prev = live.get(rect)
                if prev == key and i.sync_info is None:
                    removed += 1
                    continue
                # invalidate overlapping regions, record this load
                for r in [r for r in live if _overlaps(r, rect)]:
                    del live[r]
                live[rect] = key
            keep.append(i)
        if removed:
            bb.instructions = keep
    return removed


def widen_g32_ldweights(nc):
    """Fuse each row band's four 32-col G32 gconv LDWEIGHTS into one
    128-col load of the 4x-tiled wide block (FWL-eligible), and drop the
    narrow loads it covers.

    Emission guarantees LDW (q, c) reads cols base+32c of wg_sb where all
    four 32-col copies hold the band's diag block, so after the first LDW
    of a (band, tap) group is widened to [base, base+128) at
    tile_position (32q, 0), the later narrow loads of the same group are
    redundant (their rect already holds the right content).
    """
    widened = removed = 0
    for bb in nc.m.functions[0].blocks:
        insts = bb.instructions
        keep = []
        live = {}  # row band q -> base col of the wide block loaded
        for i in insts:
            if type(i).__name__ == "InstLdweights":
                ap = i.ins[0]
                bw = i.tile_size[0] if i.tile_size else 0
                if (ap.memref.startswith("wg_sb")
                        and i.tile_size == (32, 32)):
                    q, cc = i.tile_position
                    base = ap.offset - cc
                    if live.get(q) == base:
                        if i.sync_info is None:
                            removed += 1
                            continue
                        # carries a semaphore role: keep as narrow reload
                        # (content is correct either way)
                    else:
                        pairs = list(ap.ap)
                        assert pairs[-1] == (1, bw), pairs
                        import bass_rust
                        ap.offset = base
                        ap.ap = bass_rust.VecI64Pair(
                            [list(p) for p in pairs[:-1]] + [[1, 128]])
                        i.tile_size = (bw, 128)
                        i.tile_position = (q, 0)
                        live[q] = base
                        widened += 1
                else:
                    # any other weight load may clobber the array rows
                    live.clear()
            keep.append(i)
        if removed:
            bb.instructions = keep
    return widened, removed


def widen_pair_matmuls(nc):
    """Rewrite region-pair gconv matmuls (PSUM memref "pp*") to cover both
    regions: ifmap gains a (16*WP, 2) region dim, the PSUM out gains a
    (512, 2) bank dim.  Emission wrote the pair's lower region only."""
    import bass_rust
    widened = 0
    for bb in nc.m.functions[0].blocks:
        for i in bb.instructions:
            if type(i).__name__ != "InstMatmult":
                continue
            out = i.outs[0]
            if not out.memref.startswith("pp"):
                continue
            ifm = i.ins[0]
            ip = [list(p) for p in ifm.ap]
            op = [list(p) for p in out.ap]
            assert ip[-2:] == [[WP, ROWS_PER_TILE], [1, W]], (ip, i.name)
            assert op[-1] == [1, TILE_PX], (op, i.name)
            ifm.ap = bass_rust.VecI64Pair(
                ip[:-2] + [[16 * WP, 2]] + ip[-2:])
            out.ap = bass_rust.VecI64Pair(op[:-1] + [[512, 2]] + op[-1:])
            widened += 1
    return widened


def build_program(repeat=1):
    import contextlib

    import concourse.tile as tile
    from concourse import bacc, mybir

    f32 = mybir.dt.float32
    f16 = mybir.dt.float16
    nc = bacc.Bacc("TRN2", target_bir_lowering=False, debug=False,
                   num_devices=N_CORES)
    x_d = nc.dram_tensor("x", [BPC, C, NPIX], f16, kind="ExternalInput").ap()
    wg_d = nc.dram_tensor("wg", [128, 2 * 2 * 9 * 128], f16,
                          kind="ExternalInput").ap()
    wp_d = nc.dram_tensor("wp", [128, 2 * 2 * 2 * len(_RHOS()) * 128], f16,
                          kind="ExternalInput").ap()
    bias_d = nc.dram_tensor("bias", [128, 24], f32, kind="ExternalInput").ap()
    zeros_d = nc.dram_tensor("zeros", [128, 64], f16, kind="ExternalInput").ap()
    y_d = nc.dram_tensor("y", [BPC, C, NPIX], f16, kind="ExternalOutput").ap()

    with tile.TileContext(nc) as tc:
        with contextlib.ExitStack() as ctx:
            tc._build_ctx = ctx
            _build_body(tc, y_d, x_d, wg_d, wp_d, bias_d, zeros_d, repeat=repeat)
    widen_g32_ldweights(nc)
    if _PAIR[0]:
        widen_pair_matmuls(nc)
    prune_ldweights(nc)
    nc.compile()
    return nc


def get_program():
    key = ("nc", _REPEAT[0], _HWLOOP[0], _PW8[0], _G32[0], _TAP8[0],
           _PAIR[0], tuple(sorted(_SKIP.items())))
    if key not in _CACHED:
        _CACHED[key] = build_program(repeat=_REPEAT[0])
    return _CACHED[key]


def make_in_maps(inputs):
    wg, wp, bias = prepare_weights(inputs)
    x = np.ascontiguousarray(np.asarray(inputs["x"], np.float16))
    in_maps = []
    for i in range(N_CORES):
        in_maps.append({
            "x": x[i * BPC:(i + 1) * BPC].reshape(BPC, C, NPIX),
            "wg": wg, "wp": wp, "bias": bias,
            "zeros": np.zeros((128, 64), np.float16),
        })
    return in_maps


def _run(inputs, trace=False):
    from concourse.bass_utils import run_bass_kernel_spmd

    nc = get_program()
    in_maps = make_in_maps(inputs)
    res = run_bass_kernel_spmd(nc, in_maps, list(range(N_CORES)), trace=trace)
    out = np.concatenate(
        [res.results[i]["y"].reshape(BPC, C, H, W).astype(np.float32)
         for i in range(N_CORES)],
        axis=0)
    return out, res


def kernel(**inputs):
    return _run(inputs)[0]

